# revision 2
# baseline (speedup 1.0000x reference)
"""
Trainium2 kernel for nn_CanonicalLinear (dense_mlp).

Reference computation:
    heads[b, n, c] = x @ W[n].T + b[n]          (8 per-head linears)
    out[b, c]      = sum_n heads[b, n, c] * factor[n]

By linearity this collapses to a single linear layer:
    W_eff[c, d] = sum_n factor[n] * W[n, c, d]
    b_eff[c]    = sum_n factor[n] * b[n, c]
    out         = x @ W_eff.T + b_eff

The factor reduction is 0.06% of the matmul FLOPs, so it is folded into
the host-side weight preparation (together with the transpose to [d, c]
layout and a bf16 cast) and the device kernel is a pure streaming
matmul.  Measured per-core HBM bandwidth on this part (8 cores active)
is ~95-130 GB/s -- 3x below the cost-model figure -- so the kernel is
DMA-bound and the bf16 wire format (inputs quantized on host, fp32 PSUM
accumulation) is the main lever: per-core traffic is
  x 32/DP + weffT 8/TP + out 32/DP (fp32: 64/DP) MB.

Sharding: DP x TP grid over the 8 cores (default 4x2).  Core r =
(p, q) = divmod(r, TP) handles batch rows [p*BS, (p+1)*BS) and output
columns [q*CS, (q+1)*CS).

Per-core device kernel:
  1. DMA weffT slice [D, CS] bf16 into SBUF (16 chunk DMAs), DMA bias
     row [1, CS] and PE-broadcast it to all 128 partitions.
  2. Stream xT blocks [128, dk, BLK*128] bf16; per 128-row batch tile
     accumulate out chunks in PSUM over the 16 contraction chunks
     (bf16 matmuls, fp32 PSUM), add bias on DVE eviction, DMA out.
"""

import numpy as np
import ml_dtypes

P = 128
B, D, C, N = 8192, 2048, 2048, 8
DP, TP = 4, 2                      # data-parallel x tensor-parallel grid
BS, CS = B // DP, C // TP          # per-core batch rows / out cols
NCORES = 8

BLK = 4                            # batch tiles per x block DMA
CH = 512                           # psum chunk cols
OUT_BF16 = False

_cached_nc = None


def set_grid(dp, tp):
    global DP, TP, BS, CS, _cached_nc
    DP, TP = dp, tp
    BS, CS = B // DP, C // TP
    _cached_nc = None


def _build(bs=None, cs=None, d=D, blk=None, ch=None, out_bf16=None):
    import concourse.bass as bass
    import concourse.mybir as mybir
    import concourse.tile as tile
    from concourse import bacc

    bs = BS if bs is None else bs
    cs = CS if cs is None else cs
    blk = BLK if blk is None else blk
    ch = CH if ch is None else ch
    out_bf16 = OUT_BF16 if out_bf16 is None else out_bf16

    FP32 = mybir.dt.float32
    BF16 = mybir.dt.bfloat16
    OUT_DT = BF16 if out_bf16 else FP32

    dk = d // P                    # contraction chunks
    nbt = bs // P                  # batch tiles per core

    nc = bacc.Bacc()
    # host supplies x transposed [d, bs] and W_eff transposed [d, cs],
    # both bf16
    xd = nc.dram_tensor("x", [d, bs], BF16, kind="ExternalInput")
    wd = nc.dram_tensor("w", [d, cs], BF16, kind="ExternalInput")
    bd = nc.dram_tensor("b", [1, cs], FP32, kind="ExternalInput")
    od = nc.dram_tensor("out", [bs, cs], OUT_DT, kind="ExternalOutput")

    with tile.TileContext(nc) as tc:
        with (
            tc.tile_pool(name="singles", bufs=1) as singles,
            tc.tile_pool(name="xtp", bufs=3) as xtp,
            tc.tile_pool(name="outp", bufs=4) as outp,
            tc.tile_pool(name="psb", bufs=2, space="PSUM") as psb,
            tc.tile_pool(name="pso", bufs=6, space="PSUM") as pso,
        ):
            # --- weights: stream the whole [d, cs] slice into SBUF ------
            weffT = singles.tile([P, dk, cs], BF16)
            for k in range(dk):
                nc.sync.dma_start(weffT[:, k, :], wd[k * P:(k + 1) * P, :])

            # --- bias: load row, broadcast to 128 partitions via K=1
            # matmul with a ones column ----------------------------------
            brow = singles.tile([1, cs], FP32)
            nc.sync.dma_start(brow, bd[:])
            ones1 = singles.tile([1, P], FP32)
            nc.vector.memset(ones1, 1.0)
            beff = singles.tile([P, cs], FP32)
            for h in range(0, cs, 512):
                hw_ = min(512, cs - h)
                pw = psb.tile([P, 512], FP32, tag="pw")
                nc.tensor.matmul(pw[:, :hw_], ones1, brow[:1, h:h + hw_])
                nc.any.tensor_copy(beff[:, h:h + hw_], pw[:, :hw_])

            # --- main loop over 128-row x tiles -------------------------
            for b0 in range((nbt + blk - 1) // blk):
                nt = min(blk, nbt - b0 * blk)
                xtb = xtp.tile([P, dk, blk * P], BF16, tag="xtb")
                for k in range(dk):
                    nc.sync.dma_start(
                        xtb[:, k, :nt * P],
                        xd[k * P:(k + 1) * P,
                           b0 * blk * P:b0 * blk * P + nt * P])
                for u in range(nt):
                    osb = outp.tile([P, cs], OUT_DT)
                    for h in range(0, cs, ch):
                        hw_ = min(ch, cs - h)
                        po = pso.tile([P, ch], FP32, tag="po")
                        for k in range(dk):
                            nc.tensor.matmul(
                                po[:, :hw_],
                                xtb[:, k, u * P:(u + 1) * P],
                                weffT[:, k, h:h + hw_],
                                start=(k == 0),
                                stop=(k == dk - 1),
                            )
                        nc.vector.tensor_add(osb[:, h:h + hw_],
                                             po[:, :hw_],
                                             beff[:, h:h + hw_])
                    i = b0 * blk + u
                    nc.sync.dma_start(od[i * P:(i + 1) * P, :], osb)

    nc.finalize()
    return nc


def _get_nc():
    global _cached_nc
    if _cached_nc is None:
        _cached_nc = _build()
    return _cached_nc


def _shard_inputs(x, W, b, factor):
    # host-side weight prep: factor-reduce, transpose, quantize to bf16
    weff = np.einsum("n,ncd->cd", factor, W)          # [C, D] fp32
    weffT = np.ascontiguousarray(weff.T).astype(ml_dtypes.bfloat16)
    beff = (factor @ b).astype(np.float32)            # [C]
    in_maps = []
    xsh = {}
    for p in range(DP):
        xs = x[p * BS:(p + 1) * BS]
        xsh[p] = np.ascontiguousarray(xs.T).astype(ml_dtypes.bfloat16)
    for r in range(NCORES):
        p, q = divmod(r, TP)
        in_maps.append({
            "x": xsh[p],
            "w": np.ascontiguousarray(weffT[:, q * CS:(q + 1) * CS]),
            "b": np.ascontiguousarray(beff[None, q * CS:(q + 1) * CS]),
        })
    return in_maps


def kernel(x, W, b, factor, _trace=False):
    from concourse.bass_utils import run_bass_kernel_spmd

    x = np.asarray(x, dtype=np.float32)
    W = np.asarray(W, dtype=np.float32)
    b = np.asarray(b, dtype=np.float32)
    factor = np.asarray(factor, dtype=np.float32)

    nc = _get_nc()
    in_maps = _shard_inputs(x, W, b, factor)
    res = run_bass_kernel_spmd(nc, in_maps, list(range(NCORES)),
                               trace=_trace)

    out = np.empty((B, C), dtype=np.float32)
    for r in range(NCORES):
        p, q = divmod(r, TP)
        out[p * BS:(p + 1) * BS, q * CS:(q + 1) * CS] = \
            np.asarray(res.results[r]["out"], dtype=np.float32)
    if _trace:
        return out, res
    return out


# revision 4
# speedup vs baseline: 1.3805x; 1.3805x over previous
"""
Trainium2 kernel for nn_CanonicalLinear (dense_mlp).

Reference computation:
    heads[b, n, c] = x @ W[n].T + b[n]          (8 per-head linears)
    out[b, c]      = sum_n heads[b, n, c] * factor[n]

By linearity this collapses to a single linear layer:
    W_eff[c, d] = sum_n factor[n] * W[n, c, d]
    b_eff[c]    = sum_n factor[n] * b[n, c]
    out         = x @ W_eff.T + b_eff

The factor reduction is 0.06% of the matmul FLOPs, so it is folded into
the host-side weight preparation (together with the transpose to [d, c]
layout and a bf16 cast) and the device kernel is a pure streaming
matmul.  Measured per-core HBM bandwidth on this part (8 cores active)
is ~95-130 GB/s -- 3x below the cost-model figure -- so the kernel is
DMA-bound and the bf16 wire format (inputs quantized on host, fp32 PSUM
accumulation) is the main lever: per-core traffic is
  x 32/DP + weffT 8/TP + out 32/DP (fp32: 64/DP) MB.

Sharding: DP x TP grid over the 8 cores (default 4x2).  Core r =
(p, q) = divmod(r, TP) handles batch rows [p*BS, (p+1)*BS) and output
columns [q*CS, (q+1)*CS).

Per-core device kernel:
  1. DMA weffT slice [D, CS] bf16 into SBUF (16 chunk DMAs), DMA bias
     row [1, CS] and PE-broadcast it to all 128 partitions.
  2. Stream xT blocks [128, dk, BLK*128] bf16; per 128-row batch tile
     accumulate out chunks in PSUM over the 16 contraction chunks
     (bf16 matmuls, fp32 PSUM), add bias on DVE eviction, DMA out.
"""

import numpy as np
import ml_dtypes

P = 128
B, D, C, N = 8192, 2048, 2048, 8
DP, TP = 4, 2                      # data-parallel x tensor-parallel grid
BS, CS = B // DP, C // TP          # per-core batch rows / out cols
NCORES = 8

BLK = 4                            # batch tiles per x block DMA
CH = 512                           # psum chunk cols
OUT_BF16 = False

_cached_nc = None


def set_grid(dp, tp):
    global DP, TP, BS, CS, _cached_nc
    DP, TP = dp, tp
    BS, CS = B // DP, C // TP
    _cached_nc = None


def _build(bs=None, cs=None, d=D, blk=None, ch=None, out_bf16=None,
           wire="bf16"):
    import concourse.bass as bass
    import concourse.mybir as mybir
    import concourse.tile as tile
    from concourse import bacc

    bs = BS if bs is None else bs
    cs = CS if cs is None else cs
    blk = BLK if blk is None else blk
    ch = CH if ch is None else ch
    out_bf16 = OUT_BF16 if out_bf16 is None else out_bf16

    FP32 = mybir.dt.float32
    BF16 = mybir.dt.bfloat16
    F32R = mybir.dt.float32r
    OUT_DT = BF16 if out_bf16 else FP32
    WD = BF16 if wire == "bf16" else FP32   # dram dtype for x/w
    MMD = BF16 if wire == "bf16" else F32R  # matmul dtype

    dk = d // P                    # contraction chunks
    nbt = bs // P                  # batch tiles per core

    nc = bacc.Bacc()
    # host supplies x transposed [d, bs] and W_eff transposed [d, cs]
    xd = nc.dram_tensor("x", [d, bs], WD, kind="ExternalInput")
    wd = nc.dram_tensor("w", [d, cs], WD, kind="ExternalInput")
    bd = nc.dram_tensor("b", [1, cs], FP32, kind="ExternalInput")
    od = nc.dram_tensor("out", [bs, cs], OUT_DT, kind="ExternalOutput")

    with tile.TileContext(nc) as tc:
        with (
            tc.tile_pool(name="singles", bufs=1) as singles,
            tc.tile_pool(name="xtp", bufs=3) as xtp,
            tc.tile_pool(name="outp", bufs=4) as outp,
            tc.tile_pool(name="psb", bufs=2, space="PSUM") as psb,
            tc.tile_pool(name="pso", bufs=6, space="PSUM") as pso,
        ):
            # --- weights: stream the whole [d, cs] slice into SBUF ------
            weffT = singles.tile([P, dk, cs], MMD)
            for k in range(dk):
                nc.sync.dma_start(weffT[:, k, :],
                                  wd[k * P:(k + 1) * P, :].bitcast(MMD))

            # --- bias: load row, broadcast to 128 partitions via K=1
            # matmul with a ones column ----------------------------------
            brow = singles.tile([1, cs], FP32)
            nc.sync.dma_start(brow, bd[:])
            ones1 = singles.tile([1, P], FP32)
            nc.vector.memset(ones1, 1.0)
            beff = singles.tile([P, cs], FP32)
            for h in range(0, cs, 512):
                hw_ = min(512, cs - h)
                pw = psb.tile([P, 512], FP32, tag="pw")
                nc.tensor.matmul(pw[:, :hw_], ones1, brow[:1, h:h + hw_])
                nc.any.tensor_copy(beff[:, h:h + hw_], pw[:, :hw_])

            # --- main loop over 128-row x tiles -------------------------
            for b0 in range((nbt + blk - 1) // blk):
                nt = min(blk, nbt - b0 * blk)
                xtb = xtp.tile([P, dk, blk * P], MMD, tag="xtb")
                for k in range(dk):
                    nc.sync.dma_start(
                        xtb[:, k, :nt * P],
                        xd[k * P:(k + 1) * P,
                           b0 * blk * P:b0 * blk * P + nt * P].bitcast(MMD))
                for u in range(nt):
                    osb = outp.tile([P, cs], OUT_DT)
                    for h in range(0, cs, ch):
                        hw_ = min(ch, cs - h)
                        po = pso.tile([P, ch], FP32, tag="po")
                        for k in range(dk):
                            nc.tensor.matmul(
                                po[:, :hw_],
                                xtb[:, k, u * P:(u + 1) * P],
                                weffT[:, k, h:h + hw_],
                                start=(k == 0),
                                stop=(k == dk - 1),
                            )
                        nc.vector.tensor_add(osb[:, h:h + hw_],
                                             po[:, :hw_],
                                             beff[:, h:h + hw_])
                    i = b0 * blk + u
                    nc.sync.dma_start(od[i * P:(i + 1) * P, :], osb)

    nc.finalize()
    return nc


def _get_nc():
    global _cached_nc
    if _cached_nc is None:
        _cached_nc = _build()
    return _cached_nc


def _shard_inputs(x, W, b, factor):
    # host-side weight prep: factor-reduce, transpose, quantize to bf16
    weff = np.einsum("n,ncd->cd", factor, W)          # [C, D] fp32
    weffT = np.ascontiguousarray(weff.T).astype(ml_dtypes.bfloat16)
    beff = (factor @ b).astype(np.float32)            # [C]
    in_maps = []
    xsh = {}
    for p in range(DP):
        xs = x[p * BS:(p + 1) * BS]
        xsh[p] = np.ascontiguousarray(xs.T).astype(ml_dtypes.bfloat16)
    for r in range(NCORES):
        p, q = divmod(r, TP)
        in_maps.append({
            "x": xsh[p],
            "w": np.ascontiguousarray(weffT[:, q * CS:(q + 1) * CS]),
            "b": np.ascontiguousarray(beff[None, q * CS:(q + 1) * CS]),
        })
    return in_maps


def kernel(x, W, b, factor, _trace=False):
    from concourse.bass_utils import run_bass_kernel_spmd

    x = np.asarray(x, dtype=np.float32)
    W = np.asarray(W, dtype=np.float32)
    b = np.asarray(b, dtype=np.float32)
    factor = np.asarray(factor, dtype=np.float32)

    nc = _get_nc()
    in_maps = _shard_inputs(x, W, b, factor)
    res = run_bass_kernel_spmd(nc, in_maps, list(range(NCORES)),
                               trace=_trace)

    out = np.empty((B, C), dtype=np.float32)
    for r in range(NCORES):
        p, q = divmod(r, TP)
        out[p * BS:(p + 1) * BS, q * CS:(q + 1) * CS] = \
            np.asarray(res.results[r]["out"], dtype=np.float32)
    if _trace:
        return out, res
    return out


# revision 5
# speedup vs baseline: 1.4391x; 1.0424x over previous
"""
Trainium2 kernel for nn_CanonicalLinear (dense_mlp).

Reference computation:
    heads[b, n, c] = x @ W[n].T + b[n]          (8 per-head linears)
    out[b, c]      = sum_n heads[b, n, c] * factor[n]

By linearity this collapses to a single linear layer:
    W_eff[c, d] = sum_n factor[n] * W[n, c, d]
    b_eff[c]    = sum_n factor[n] * b[n, c]
    out         = x @ W_eff.T + b_eff

The factor reduction is 0.06% of the matmul FLOPs, so it is folded into
the host-side weight preparation and the device kernel is a pure
streaming matmul.

Measured DMA behavior on this part (8 cores active): transfer cost is
dominated by per-descriptor issue (~80ns per SBUF partition row),
nearly independent of row size up to ~16-32KB.  So all tensors are
pre-tiled on the host into [128, huge-row] layouts where one DMA moves
2-4MB with 16-32KB descriptors, and bf16 payloads are DMA'd under an
fp32 dtype (bitcast to bf16 only at matmul use) since 2-byte-typed
DMAs measure ~2x slower.

Sharding: DP x TP grid over the 8 cores.  Core r = (p, q) =
divmod(r, TP) handles batch rows [p*BS, (p+1)*BS) and output columns
[q*CS, (q+1)*CS).

Device kernel per core:
  1. one DMA for the weffT slice ([128, dk*CS] partition-major tiled),
     bias row DMA + PE broadcast to 128 partitions.
  2. per block of BLK batch tiles: one x DMA [128, dk*BLK*128]; bf16
     matmuls accumulate [128, 512] PSUM chunks over dk=16 contraction
     chunks; DVE adds bias on eviction into a block output buffer;
     one out DMA [128, BLK*CS] per block.
"""

import numpy as np
import ml_dtypes

P = 128
B, D, C, N = 8192, 2048, 2048, 8
DP, TP = 4, 2                      # data-parallel x tensor-parallel grid
BS, CS = B // DP, C // TP          # per-core batch rows / out cols
NCORES = 8
DK = D // P

BLK = 4                            # batch tiles per x/out block DMA
CH = 512                           # psum chunk cols
OUT_BF16 = False
WIRE = "bf16"

_cached_nc = None


def set_grid(dp, tp):
    global DP, TP, BS, CS, _cached_nc
    DP, TP = dp, tp
    BS, CS = B // DP, C // TP
    _cached_nc = None


def _build(bs=None, cs=None, d=D, blk=None, ch=None, out_bf16=None,
           wire=None):
    import concourse.bass as bass
    import concourse.mybir as mybir
    import concourse.tile as tile
    from concourse import bacc

    bs = BS if bs is None else bs
    cs = CS if cs is None else cs
    blk = BLK if blk is None else blk
    ch = CH if ch is None else ch
    out_bf16 = OUT_BF16 if out_bf16 is None else out_bf16
    wire = WIRE if wire is None else wire

    FP32 = mybir.dt.float32
    BF16 = mybir.dt.bfloat16
    F32R = mybir.dt.float32r
    OUT_DT = BF16 if out_bf16 else FP32
    MMD = BF16 if wire == "bf16" else F32R  # matmul dtype
    esz = 2 if wire == "bf16" else 4        # payload element size
    ew = esz // 2                           # fp32 words per 2 payload elems
    # payload columns per fp32 word
    cpw = 4 // esz

    dk = d // P                    # contraction chunks
    nbt = bs // P                  # batch tiles per core
    nblk = (nbt + blk - 1) // blk

    nc = bacc.Bacc()
    # tiled, partition-major layouts; payload bf16 viewed as fp32 words
    xd = nc.dram_tensor("x", [nblk, P, dk * blk * P // cpw], FP32,
                        kind="ExternalInput")
    wd = nc.dram_tensor("w", [P, dk * cs // cpw], FP32,
                        kind="ExternalInput")
    bd = nc.dram_tensor("b", [1, cs], FP32, kind="ExternalInput")
    od = nc.dram_tensor("out", [nblk, P, blk * cs], OUT_DT,
                        kind="ExternalOutput")

    with tile.TileContext(nc) as tc:
        with (
            tc.tile_pool(name="singles", bufs=1) as singles,
            tc.tile_pool(name="xtp", bufs=3) as xtp,
            tc.tile_pool(name="outp", bufs=3) as outp,
            tc.tile_pool(name="psb", bufs=2, space="PSUM") as psb,
            tc.tile_pool(name="pso", bufs=6, space="PSUM") as pso,
        ):
            # --- weights: one big DMA (fp32-typed words) ----------------
            wsb = singles.tile([P, dk, cs // cpw], FP32)
            hw_half = dk // 2
            for wh in range(2):
                nc.sync.dma_start(
                    wsb[:, wh * hw_half:(wh + 1) * hw_half, :],
                    wd[:, wh * (dk * cs // cpw // 2):
                       (wh + 1) * (dk * cs // cpw // 2)])

            # --- bias: load row, broadcast to 128 partitions via K=1
            # matmul with a ones column ----------------------------------
            brow = singles.tile([1, cs], FP32)
            nc.sync.dma_start(brow, bd[:])
            ones1 = singles.tile([1, P], FP32)
            nc.vector.memset(ones1, 1.0)
            beff = singles.tile([P, cs], FP32)
            for h in range(0, cs, 512):
                hw_ = min(512, cs - h)
                pw = psb.tile([P, 512], FP32, tag="pw")
                nc.tensor.matmul(pw[:, :hw_], ones1, brow[:1, h:h + hw_])
                nc.any.tensor_copy(beff[:, h:h + hw_], pw[:, :hw_])

            # --- main loop over BLK-tile blocks -------------------------
            for b0 in range(nblk):
                nt = min(blk, nbt - b0 * blk)
                xtb = xtp.tile([P, dk, blk * P // cpw], FP32, tag="xtb")
                nc.sync.dma_start(
                    xtb[:, :, :] if nt == blk else xtb[:, :, :nt * P // cpw],
                    xd[b0, :, :] if nt == blk
                    else xd[b0, :, :dk * nt * P // cpw])
                osb = outp.tile([P, blk, cs], OUT_DT)
                for u in range(nt):
                    for h in range(0, cs, ch):
                        hw_ = min(ch, cs - h)
                        po = pso.tile([P, ch], FP32, tag="po")
                        for k in range(dk):
                            nc.tensor.matmul(
                                po[:, :hw_],
                                xtb[:, k,
                                    u * P // cpw:(u + 1) * P // cpw]
                                .bitcast(MMD),
                                wsb[:, k, h // cpw:(h + hw_) // cpw]
                                .bitcast(MMD),
                                start=(k == 0),
                                stop=(k == dk - 1),
                            )
                        nc.vector.tensor_add(osb[:, u, h:h + hw_],
                                             po[:, :hw_],
                                             beff[:, h:h + hw_])
                nc.sync.dma_start(
                    od[b0, :, :] if nt == blk else od[b0, :, :nt * cs],
                    osb[:, :, :] if nt == blk else osb[:, :nt, :])

    nc.finalize()
    return nc


def _get_nc():
    global _cached_nc
    if _cached_nc is None:
        _cached_nc = _build()
    return _cached_nc


def _tile_w(weffT_cs, dk, cs, esz):
    # [D, cs] payload -> [128, dk, cs] partition-major -> fp32-word view
    a = np.ascontiguousarray(
        weffT_cs.reshape(dk, P, cs).transpose(1, 0, 2))
    return a.reshape(P, dk * cs).view(np.float32)


def _tile_x(xT, dk, blk, nblk, esz):
    # [D, BS] payload -> [nblk, 128, dk, blk*128] -> fp32-word view
    bs = xT.shape[1]
    a = xT.reshape(dk, P, nblk, blk * P).transpose(2, 1, 0, 3)
    a = np.ascontiguousarray(a)
    return a.reshape(nblk, P, dk * blk * P).view(np.float32)


def _shard_inputs(x, W, b, factor):
    wdt = ml_dtypes.bfloat16 if WIRE == "bf16" else np.float32
    esz = 2 if WIRE == "bf16" else 4
    nbt = BS // P
    nblk = (nbt + BLK - 1) // BLK
    # host-side weight prep: factor-reduce, transpose, quantize
    weff = np.einsum("n,ncd->cd", factor, W)          # [C, D] fp32
    weffT = np.ascontiguousarray(weff.T).astype(wdt)  # [D, C]
    beff = (factor @ b).astype(np.float32)            # [C]
    in_maps = []
    xsh = {}
    for p in range(DP):
        xs = x[p * BS:(p + 1) * BS]
        xT = np.ascontiguousarray(xs.T).astype(wdt)
        xsh[p] = _tile_x(xT, DK, BLK, nblk, esz)
    wq = {}
    for q in range(TP):
        wq[q] = _tile_w(
            np.ascontiguousarray(weffT[:, q * CS:(q + 1) * CS]),
            DK, CS, esz)
    for r in range(NCORES):
        p, q = divmod(r, TP)
        in_maps.append({
            "x": xsh[p],
            "w": wq[q],
            "b": np.ascontiguousarray(beff[None, q * CS:(q + 1) * CS]),
        })
    return in_maps


def _assemble(res_out_list):
    """res_out_list[r] = out array [nblk, 128, blk*cs] -> full [B, C]."""
    nbt = BS // P
    nblk = (nbt + BLK - 1) // BLK
    out = np.empty((B, C), dtype=np.float32)
    for r in range(NCORES):
        p, q = divmod(r, TP)
        oc = np.asarray(res_out_list[r], dtype=np.float32)
        oc = oc.reshape(nblk, P, BLK, CS).transpose(0, 2, 1, 3)
        oc = oc.reshape(BS, CS)
        out[p * BS:(p + 1) * BS, q * CS:(q + 1) * CS] = oc
    return out


def kernel(x, W, b, factor, _trace=False):
    from concourse.bass_utils import run_bass_kernel_spmd

    x = np.asarray(x, dtype=np.float32)
    W = np.asarray(W, dtype=np.float32)
    b = np.asarray(b, dtype=np.float32)
    factor = np.asarray(factor, dtype=np.float32)

    nc = _get_nc()
    in_maps = _shard_inputs(x, W, b, factor)
    res = run_bass_kernel_spmd(nc, in_maps, list(range(NCORES)),
                               trace=_trace)

    out = _assemble([res.results[r]["out"] for r in range(NCORES)])
    if _trace:
        return out, res
    return out


# revision 7
# speedup vs baseline: 2.0900x; 1.4523x over previous
"""
Trainium2 kernel for nn_CanonicalLinear (dense_mlp).

Reference computation:
    heads[b, n, c] = x @ W[n].T + b[n]          (8 per-head linears)
    out[b, c]      = sum_n heads[b, n, c] * factor[n]

By linearity this collapses to a single linear layer:
    W_eff[c, d] = sum_n factor[n] * W[n, c, d]
    b_eff[c]    = sum_n factor[n] * b[n, c]
    out         = x @ W_eff.T + b_eff

The factor reduction is 0.06% of the matmul FLOPs, so it is folded into
the host-side weight preparation and the device kernel is a pure
streaming matmul.

Measured DMA behavior on this part (8 cores active): transfer cost is
dominated by per-descriptor issue (~80ns per SBUF partition row),
nearly independent of row size up to ~16-32KB.  So all tensors are
pre-tiled on the host into [128, huge-row] layouts where one DMA moves
2-4MB with 16-32KB descriptors, and bf16 payloads are DMA'd under an
fp32 dtype (bitcast to bf16 only at matmul use) since 2-byte-typed
DMAs measure ~2x slower.

Sharding: DP x TP grid over the 8 cores.  Core r = (p, q) =
divmod(r, TP) handles batch rows [p*BS, (p+1)*BS) and output columns
[q*CS, (q+1)*CS).

Device kernel per core:
  1. one DMA for the weffT slice ([128, dk*CS] partition-major tiled),
     bias row DMA + PE broadcast to 128 partitions.
  2. per block of BLK batch tiles: one x DMA [128, dk*BLK*128]; bf16
     matmuls accumulate [128, 512] PSUM chunks over dk=16 contraction
     chunks; DVE adds bias on eviction into a block output buffer;
     one out DMA [128, BLK*CS] per block.
"""

import numpy as np
import ml_dtypes

P = 128
B, D, C, N = 8192, 2048, 2048, 8
DP, TP = 4, 2                      # data-parallel x tensor-parallel grid
BS, CS = B // DP, C // TP          # per-core batch rows / out cols
NCORES = 8
DK = D // P

BLK = 4                            # batch tiles per x/out block DMA
CH = 512                           # psum chunk cols
OUT_BF16 = False
WIRE = "bf16"

_cached_nc = None


def set_grid(dp, tp):
    global DP, TP, BS, CS, _cached_nc
    DP, TP = dp, tp
    BS, CS = B // DP, C // TP
    _cached_nc = None


def _build(bs=None, cs=None, d=D, blk=None, ch=None, out_bf16=None,
           wire=None):
    import concourse.bass as bass
    import concourse.mybir as mybir
    import concourse.tile as tile
    from concourse import bacc

    bs = BS if bs is None else bs
    cs = CS if cs is None else cs
    blk = BLK if blk is None else blk
    ch = CH if ch is None else ch
    out_bf16 = OUT_BF16 if out_bf16 is None else out_bf16
    wire = WIRE if wire is None else wire

    FP32 = mybir.dt.float32
    BF16 = mybir.dt.bfloat16
    F32R = mybir.dt.float32r
    OUT_DT = BF16 if out_bf16 else FP32
    MMD = BF16 if wire == "bf16" else F32R  # matmul dtype
    esz = 2 if wire == "bf16" else 4        # payload element size
    ew = esz // 2                           # fp32 words per 2 payload elems
    # payload columns per fp32 word
    cpw = 4 // esz

    dk = d // P                    # contraction chunks
    nbt = bs // P                  # batch tiles per core
    nblk = (nbt + blk - 1) // blk

    nc = bacc.Bacc()
    # tiled, partition-major layouts; payload bf16 viewed as fp32 words
    xd = nc.dram_tensor("x", [nblk, P, dk * blk * P // cpw], FP32,
                        kind="ExternalInput")
    wd = nc.dram_tensor("w", [P, dk * cs // cpw], FP32,
                        kind="ExternalInput")
    bd = nc.dram_tensor("b", [1, cs], FP32, kind="ExternalInput")
    od = nc.dram_tensor("out", [nblk, P, blk * cs], OUT_DT,
                        kind="ExternalOutput")

    with tile.TileContext(nc) as tc:
        with (
            tc.tile_pool(name="singles", bufs=1) as singles,
            tc.tile_pool(name="xtp", bufs=3 if esz == 2 else 2) as xtp,
            tc.tile_pool(name="outp", bufs=3 if esz == 2 else 2) as outp,
            tc.tile_pool(name="psb", bufs=2, space="PSUM") as psb,
            tc.tile_pool(name="pso", bufs=6, space="PSUM") as pso,
        ):
            # --- weights: one big DMA (fp32-typed words) ----------------
            TD = FP32 if esz == 2 else F32R
            wsb = singles.tile([P, dk, cs // cpw], TD)
            hw_half = dk // 2
            for wh in range(2):
                src_ap = wd[:, wh * (dk * cs // cpw // 2):
                            (wh + 1) * (dk * cs // cpw // 2)]
                if esz == 4:
                    src_ap = src_ap.bitcast(F32R)
                nc.sync.dma_start(
                    wsb[:, wh * hw_half:(wh + 1) * hw_half, :], src_ap)

            # --- bias: load row, broadcast to 128 partitions via K=1
            # matmul with a ones column ----------------------------------
            brow = singles.tile([1, cs], FP32)
            nc.sync.dma_start(brow, bd[:])
            ones1 = singles.tile([1, P], FP32)
            nc.vector.memset(ones1, 1.0)
            beff = singles.tile([P, cs], FP32)
            for h in range(0, cs, 512):
                hw_ = min(512, cs - h)
                pw = psb.tile([P, 512], FP32, tag="pw")
                nc.tensor.matmul(pw[:, :hw_], ones1, brow[:1, h:h + hw_])
                nc.any.tensor_copy(beff[:, h:h + hw_], pw[:, :hw_])

            # --- main loop over BLK-tile blocks -------------------------
            for b0 in range(nblk):
                nt = min(blk, nbt - b0 * blk)
                xtb = xtp.tile([P, dk, blk * P // cpw], TD, tag="xtb")
                xsrc = (xd[b0, :, :] if nt == blk
                        else xd[b0, :, :dk * nt * P // cpw])
                if esz == 4:
                    xsrc = xsrc.bitcast(F32R)
                nc.sync.dma_start(
                    xtb[:, :, :] if nt == blk else xtb[:, :, :nt * P // cpw],
                    xsrc)
                osb = outp.tile([P, blk, cs], OUT_DT)
                for u in range(nt):
                    for h in range(0, cs, ch):
                        hw_ = min(ch, cs - h)
                        po = pso.tile([P, ch], FP32, tag="po")
                        for k in range(dk):
                            nc.tensor.matmul(
                                po[:, :hw_],
                                xtb[:, k,
                                    u * P // cpw:(u + 1) * P // cpw]
                                .bitcast(MMD) if esz == 2 else
                                xtb[:, k, u * P:(u + 1) * P],
                                wsb[:, k, h // cpw:(h + hw_) // cpw]
                                .bitcast(MMD) if esz == 2 else
                                wsb[:, k, h:h + hw_],
                                start=(k == 0),
                                stop=(k == dk - 1),
                            )
                        nc.vector.tensor_add(osb[:, u, h:h + hw_],
                                             po[:, :hw_],
                                             beff[:, h:h + hw_])
                nc.sync.dma_start(
                    od[b0, :, :] if nt == blk else od[b0, :, :nt * cs],
                    osb[:, :, :] if nt == blk else osb[:, :nt, :])

    nc.finalize()
    return nc


def _get_nc():
    global _cached_nc
    if _cached_nc is None:
        _cached_nc = _build()
    return _cached_nc


def _tile_w(weffT_cs, dk, cs, esz):
    # [D, cs] payload -> [128, dk, cs] partition-major -> fp32-word view
    a = np.ascontiguousarray(
        weffT_cs.reshape(dk, P, cs).transpose(1, 0, 2))
    return a.reshape(P, dk * cs).view(np.float32)


def _tile_x(xT, dk, blk, nblk, esz):
    # [D, BS] payload -> [nblk, 128, dk, blk*128] -> fp32-word view
    bs = xT.shape[1]
    a = xT.reshape(dk, P, nblk, blk * P).transpose(2, 1, 0, 3)
    a = np.ascontiguousarray(a)
    return a.reshape(nblk, P, dk * blk * P).view(np.float32)


def _shard_inputs(x, W, b, factor):
    wdt = ml_dtypes.bfloat16 if WIRE == "bf16" else np.float32
    esz = 2 if WIRE == "bf16" else 4
    nbt = BS // P
    nblk = (nbt + BLK - 1) // BLK
    # host-side weight prep: factor-reduce, transpose, quantize
    weff = np.einsum("n,ncd->cd", factor, W)          # [C, D] fp32
    weffT = np.ascontiguousarray(weff.T).astype(wdt)  # [D, C]
    beff = (factor @ b).astype(np.float32)            # [C]
    in_maps = []
    xsh = {}
    for p in range(DP):
        xs = x[p * BS:(p + 1) * BS]
        xT = np.ascontiguousarray(xs.T).astype(wdt)
        xsh[p] = _tile_x(xT, DK, BLK, nblk, esz)
    wq = {}
    for q in range(TP):
        wq[q] = _tile_w(
            np.ascontiguousarray(weffT[:, q * CS:(q + 1) * CS]),
            DK, CS, esz)
    for r in range(NCORES):
        p, q = divmod(r, TP)
        in_maps.append({
            "x": xsh[p],
            "w": wq[q],
            "b": np.ascontiguousarray(beff[None, q * CS:(q + 1) * CS]),
        })
    return in_maps


def _assemble(res_out_list):
    """res_out_list[r] = out array [nblk, 128, blk*cs] -> full [B, C]."""
    nbt = BS // P
    nblk = (nbt + BLK - 1) // BLK
    out = np.empty((B, C), dtype=np.float32)
    for r in range(NCORES):
        p, q = divmod(r, TP)
        oc = np.asarray(res_out_list[r], dtype=np.float32)
        oc = oc.reshape(nblk, P, BLK, CS).transpose(0, 2, 1, 3)
        oc = oc.reshape(BS, CS)
        out[p * BS:(p + 1) * BS, q * CS:(q + 1) * CS] = oc
    return out


def kernel(x, W, b, factor, _trace=False):
    from concourse.bass_utils import run_bass_kernel_spmd

    x = np.asarray(x, dtype=np.float32)
    W = np.asarray(W, dtype=np.float32)
    b = np.asarray(b, dtype=np.float32)
    factor = np.asarray(factor, dtype=np.float32)

    nc = _get_nc()
    in_maps = _shard_inputs(x, W, b, factor)
    res = run_bass_kernel_spmd(nc, in_maps, list(range(NCORES)),
                               trace=_trace)

    out = _assemble([res.results[r]["out"] for r in range(NCORES)])
    if _trace:
        return out, res
    return out
